# revision 3
# baseline (speedup 1.0000x reference)
"""Trainium2 Bass kernel for MoE-routed LoRA model (nn_LoraModel_51677046505966).

Math (per reference):
  base  = x @ W.T + bias
  gate  = x @ route_W.T                         (fp32, output)
  probs = softmax(gate); top-2 renormalized -> routing_weight (dense, output)
  h     = x @ lora_A.T  per expert              -> gated by routing_weight
  lora  = sum_e w_e * (h_e @ lora_B_e.T)
  result = base + lora * (lora_alpha / r)       (scaling = 2.0, folded into lora_B)

Strategy: data-parallel over the 8192 tokens across 8 NeuronCores (1024
tokens/core; weights replicated). All device matmuls run "feature-major":
out[features_on_partitions, tokens_on_free], with pre-transposed weights
shipped from the host so every DMA is a plain strided read. x is streamed
once in fp32 (feeds the exact-fp32 router matmul, whose top-2 selection
needs ~1e-5 logit accuracy), then cast on-device to fp16 for the
base/LoRA matmuls (fp16 runs at 1 cycle/row on the PE like bf16, with
2^-11 rounding instead of bf16's 2^-8). Outputs are written transposed
([d_out, tokens]) so DMA writes stay contiguous; the host transposes back.
"""
import numpy as np
import ml_dtypes

import concourse.bass as bass
import concourse.bacc as bacc
import concourse.tile as tile
from concourse import mybir
from concourse.bass_utils import run_bass_kernel_spmd

dt = mybir.dt

# Problem dims (hardcoded per spec)
B, S, D_IN, D_OUT, E, R = 4, 2048, 4096, 4096, 8, 16
N_CORES = 8
TOKENS = B * S
SCALING = 2.0  # lora_alpha / r = 32 / 16
P = 128
ER = E * R  # 128


def build_nc(d_in=D_IN, d_out=D_OUT, tpc=TOKENS // N_CORES, tt=512, mm_dt=dt.float16,
             repeat=1):
    """Build the per-core Bass program. All cores run the same program (SPMD).

    repeat > 1 duplicates the whole compute (same outputs) for wall-clock
    slope timing: delta(wall)/delta(repeat) = per-iteration HW time.
    """
    KC = d_in // P        # contraction chunks
    NT = tpc // tt        # token tiles per core
    OC2 = d_out // P      # 128-wide output chunks
    OB = d_out // 256     # 256-wide output blocks (W stream granularity)
    NC4 = tt // P         # 128-token subchunks per token tile

    nc = bacc.Bacc(target_bir_lowering=False)

    xT = nc.dram_tensor("xT", [d_in, tpc], dt.float32, kind="ExternalInput")
    WT2 = nc.dram_tensor("WT2", [OB, KC, P, 256], mm_dt, kind="ExternalInput")
    lat = nc.dram_tensor("lat", [KC, P, ER], mm_dt, kind="ExternalInput")
    lb2 = nc.dram_tensor("lb2", [ER, d_out], mm_dt, kind="ExternalInput")
    rwt = nc.dram_tensor("rwt", [KC, P, E], dt.float32, kind="ExternalInput")
    biasT = nc.dram_tensor("biasT", [P, OC2], dt.float32, kind="ExternalInput")
    eye8 = nc.dram_tensor("eye8", [E, E], dt.float32, kind="ExternalInput")
    eye128 = nc.dram_tensor("eye128", [P, P], dt.float32, kind="ExternalInput")
    expM = nc.dram_tensor("expM", [E, ER], dt.float32, kind="ExternalInput")

    resT = nc.dram_tensor("resT", [d_out, tpc], dt.float32, kind="ExternalOutput")
    gate_o = nc.dram_tensor("gate_o", [tpc, E], dt.float32, kind="ExternalOutput")
    rw_o = nc.dram_tensor("rw_o", [tpc, E], dt.float32, kind="ExternalOutput")

    gate_o_v = gate_o.rearrange("(n c p) e -> n p c e", c=NC4, p=P)
    rw_o_v = rw_o.rearrange("(n c p) e -> n p c e", c=NC4, p=P)
    xT_v = xT.rearrange("(k p) (n t) -> k p n t", p=P, t=tt)

    X = mybir.AxisListType.X
    Alu = mybir.AluOpType

    with tile.TileContext(nc) as tc:
        with (
            tc.tile_pool(name="consts", bufs=1) as consts,
            tc.tile_pool(name="xcache", bufs=1) as xcache,
            tc.tile_pool(name="xpool", bufs=3) as xpool,
            tc.tile_pool(name="wpool", bufs=2) as wpool,
            tc.tile_pool(name="opool", bufs=3) as opool,
            tc.tile_pool(name="rpool", bufs=2) as rpool,
            tc.tile_pool(name="psum", bufs=2, space="PSUM") as psum,
            tc.tile_pool(name="psA", bufs=1, space="PSUM") as psA,
        ):
            lat_sb = consts.tile([P, KC, ER], mm_dt)
            nc.sync.dma_start(lat_sb[:], lat.rearrange("k p e -> p k e"))
            lb2_sb = consts.tile([ER, d_out], mm_dt)
            nc.sync.dma_start(lb2_sb[:], lb2[:])
            rwt_sb = consts.tile([P, KC, E], dt.float32)
            nc.sync.dma_start(rwt_sb[:], rwt.rearrange("k p e -> p k e"))
            biasT_sb = consts.tile([P, OC2], dt.float32)
            nc.sync.dma_start(biasT_sb[:], biasT[:])
            eye8_sb = consts.tile([E, E], dt.float32)
            nc.sync.dma_start(eye8_sb[:], eye8[:])
            eye128_sb = consts.tile([P, P], dt.float32)
            nc.sync.dma_start(eye128_sb[:], eye128[:])
            expM_sb = consts.tile([E, ER], dt.float32)
            nc.sync.dma_start(expM_sb[:], expM[:])

            xr = xcache.tile([P, KC, NT, tt], mm_dt)
            hw = xcache.tile([P, NT, tt], mm_dt)

            # ---- Phase A: router (exact fp32) + LoRA-A + gating, per token tile
            for n in range(NT):
                gate_ps = psA.tile([E, tt], dt.float32, tag="gate")
                for k in range(KC):
                    xt = xpool.tile([P, tt], dt.float32, tag="xt")
                    nc.sync.dma_start(xt[:], xT_v[k, :, n, :])
                    nc.tensor.matmul(gate_ps[:], rwt_sb[:, k, :], xt[:],
                                     start=(k == 0), stop=(k == KC - 1))
                    nc.vector.tensor_copy(xr[:, k, n, :], xt[:])  # fp32 -> fp16

                h_ps = psA.tile([P, tt], dt.float32, tag="h")
                for k in range(KC):
                    nc.tensor.matmul(h_ps[:], lat_sb[:, k, :], xr[:, k, n, :],
                                     start=(k == 0), stop=(k == KC - 1))

                gate_sb = rpool.tile([E, tt], dt.float32, tag="gate_sb")
                nc.vector.tensor_copy(gate_sb[:], gate_ps[:])

                # transpose gate to token-major [P, NC4, E]
                gtp = psA.tile([P, NC4, E], dt.float32, tag="gtp")
                for c in range(NC4):
                    nc.tensor.transpose(gtp[:, c, :], gate_sb[:, c * P:(c + 1) * P],
                                        eye8_sb[:])
                g_tm = rpool.tile([P, NC4, E], dt.float32, tag="g_tm")
                nc.vector.tensor_copy(g_tm[:], gtp[:])
                nc.sync.dma_start(gate_o_v[n], g_tm[:])

                # top-2 renormalized routing weights, token-major
                m1 = rpool.tile([P, NC4, 1], dt.float32, tag="m1")
                nc.vector.tensor_reduce(m1[:], g_tm[:], axis=X, op=Alu.max)
                eq = rpool.tile([P, NC4, E], dt.float32, tag="eq")
                nc.vector.tensor_tensor(eq[:], g_tm[:],
                                        m1.broadcast_to((P, NC4, E)),
                                        op=Alu.is_equal)
                masked = rpool.tile([P, NC4, E], dt.float32, tag="masked")
                nc.vector.scalar_tensor_tensor(masked[:], eq[:], -1e30, g_tm[:],
                                               op0=Alu.mult, op1=Alu.add)
                m2 = rpool.tile([P, NC4, 1], dt.float32, tag="m2")
                nc.vector.tensor_reduce(m2[:], masked[:], axis=X, op=Alu.max)
                sel = rpool.tile([P, NC4, E], dt.float32, tag="sel")
                nc.vector.tensor_tensor(sel[:], g_tm[:],
                                        m2.broadcast_to((P, NC4, E)), op=Alu.is_ge)
                gs = rpool.tile([P, NC4, E], dt.float32, tag="gs")
                nc.vector.tensor_tensor(gs[:], g_tm[:],
                                        m1.broadcast_to((P, NC4, E)), op=Alu.subtract)
                ex = rpool.tile([P, NC4, E], dt.float32, tag="ex")
                nc.scalar.activation(ex[:], gs[:], mybir.ActivationFunctionType.Exp)
                es = rpool.tile([P, NC4, E], dt.float32, tag="es")
                nc.vector.tensor_mul(es[:], ex[:], sel[:])
                ssum = rpool.tile([P, NC4, 1], dt.float32, tag="ssum")
                nc.vector.tensor_reduce(ssum[:], es[:], axis=X, op=Alu.add)
                rcp = rpool.tile([P, NC4, 1], dt.float32, tag="rcp")
                nc.vector.reciprocal(rcp[:], ssum[:])
                rw_tm = rpool.tile([P, NC4, E], dt.float32, tag="rw_tm")
                nc.vector.tensor_tensor(rw_tm[:], es[:],
                                        rcp.broadcast_to((P, NC4, E)), op=Alu.mult)
                nc.sync.dma_start(rw_o_v[n], rw_tm[:])

                # back to feature-major: w^T [E, tt]
                wtp = psA.tile([E, NC4, P], dt.float32, tag="wtp")
                for c in range(NC4):
                    nc.tensor.transpose(wtp[:, c, :], rw_tm[:, c, :], eye128_sb[:])
                wT_sb = rpool.tile([E, NC4 * P], dt.float32, tag="wT_sb")
                nc.vector.tensor_copy(wT_sb[:], wtp.rearrange("e c p -> e (c p)"))

                # expand per-expert weights to the 128 (e, r) rows: wexp = expM.T @ wT
                wexp_ps = psA.tile([P, tt], dt.float32, tag="wexp")
                nc.tensor.matmul(wexp_ps[:], expM_sb[:], wT_sb[:],
                                 start=True, stop=True)

                h_sb = rpool.tile([P, tt], dt.float32, tag="h_sb")
                nc.scalar.copy(h_sb[:], h_ps[:])
                nc.vector.tensor_tensor(hw[:, n, :], h_sb[:], wexp_ps[:],
                                        op=Alu.mult)  # -> fp16

            # ---- Phase B: base matmul + LoRA-B accumulation, stream W once
            for ob in range(OB):
                wt = wpool.tile([P, KC, 256], mm_dt, tag="wt")
                nc.sync.dma_start(wt[:], WT2[ob].rearrange("k p o -> p k o"))
                for oj in range(2):
                    oc = ob * 2 + oj
                    for n in range(NT):
                        acc = psum.tile([P, tt], dt.float32, tag="acc")
                        for k in range(KC):
                            nc.tensor.matmul(acc[:], wt[:, k, oj * P:(oj + 1) * P],
                                             xr[:, k, n, :],
                                             start=(k == 0), stop=False)
                        nc.tensor.matmul(acc[:], lb2_sb[:, oc * P:(oc + 1) * P],
                                         hw[:, n, :], start=False, stop=True)
                        out_sb = opool.tile([P, tt], dt.float32, tag="out")
                        nc.vector.scalar_tensor_tensor(
                            out_sb[:], acc[:], 1.0,
                            biasT_sb[:, oc:oc + 1].broadcast_to((P, tt)),
                            op0=Alu.mult, op1=Alu.add)
                        nc.sync.dma_start(
                            resT[oc * P:(oc + 1) * P, n * tt:(n + 1) * tt], out_sb[:])

    nc.compile()
    return nc


def host_inputs(x, W, bias, route_W, lora_A, lora_B,
                d_in=D_IN, d_out=D_OUT, tpc=TOKENS // N_CORES, n_cores=N_CORES,
                np_mm=np.float16):
    """Shard + lay out host-side inputs for each core."""
    KC = d_in // P
    OC2 = d_out // P
    OB = d_out // 256

    x2d = np.ascontiguousarray(x.reshape(-1, d_in).astype(np.float32))
    WT = np.ascontiguousarray(W.astype(np_mm).T)          # [d_in, d_out]
    WT2 = np.ascontiguousarray(
        WT.reshape(KC, P, OB, 256).transpose(2, 0, 1, 3))  # [OB, KC, P, 256]
    lat = np.ascontiguousarray(
        lora_A.reshape(ER, d_in).astype(np_mm).T.reshape(KC, P, ER))
    lb2 = np.ascontiguousarray(
        (lora_B.transpose(0, 2, 1).reshape(ER, d_out) * SCALING).astype(np_mm))
    rwt = np.ascontiguousarray(
        route_W.astype(np.float32).T.reshape(KC, P, E))
    biasT = np.ascontiguousarray(bias.astype(np.float32).reshape(OC2, P).T)
    eye8 = np.eye(E, dtype=np.float32)
    eye128 = np.eye(P, dtype=np.float32)
    expM = np.repeat(np.eye(E, dtype=np.float32), R, axis=1)  # [E, ER]

    in_maps = []
    for c in range(n_cores):
        xT = np.ascontiguousarray(x2d[c * tpc:(c + 1) * tpc].T)
        in_maps.append({
            "xT": xT, "WT2": WT2, "lat": lat, "lb2": lb2, "rwt": rwt,
            "biasT": biasT, "eye8": eye8, "eye128": eye128, "expM": expM,
        })
    return in_maps


_NC_CACHE = {}


def kernel(x, W, bias, route_W, lora_A, lora_B):
    x = np.asarray(x)
    W = np.asarray(W)
    bias = np.asarray(bias)
    route_W = np.asarray(route_W)
    lora_A = np.asarray(lora_A)
    lora_B = np.asarray(lora_B)

    tpc = TOKENS // N_CORES
    if "nc" not in _NC_CACHE:
        _NC_CACHE["nc"] = build_nc()
    nc = _NC_CACHE["nc"]

    in_maps = host_inputs(x, W, bias, route_W, lora_A, lora_B)
    res = run_bass_kernel_spmd(nc, in_maps, core_ids=list(range(N_CORES)))

    result = np.empty((TOKENS, D_OUT), dtype=np.float32)
    gate = np.empty((TOKENS, E), dtype=np.float32)
    rw = np.empty((TOKENS, E), dtype=np.float32)
    for c in range(N_CORES):
        o = res.results[c]
        result[c * tpc:(c + 1) * tpc] = o["resT"].T
        gate[c * tpc:(c + 1) * tpc] = o["gate_o"]
        rw[c * tpc:(c + 1) * tpc] = o["rw_o"]

    return (result.reshape(B, S, D_OUT),
            rw.reshape(B, S, E),
            gate.reshape(B, S, E))
